# revision 39
# baseline (speedup 1.0000x reference)
"""EvidenceNet pairwise-MLP scoring kernel for 8 Trainium2 NeuronCores.

Math (reference):
    img = sign(images_hash)/8, txt = sign(texts_hash)/8          [1024, 64] each
    a[i,k] = (img @ W1[:, :64].T)[i,k] + b1[k]                   [1024, 128]
    t[j,k] = (txt @ W1[:, 64:].T)[j,k]                           [1024, 128]
    negE[i,j] = sum_k W2[0,k] * relu(a[i,k] + t[j,k]) + b2[0]
    posE[i,j] = img[i,:] @ txt[j,:]
    out = [exp(clip(posE/0.5)), exp(clip(negE/0.5))] flattened   [1024*1024, 2]
    (clip at +-15 never binds: |2*negE| < 1, |2*posE| <= 2)

Distribution: data-parallel over image rows; core c owns i in [128c, 128c+128).

Per-core device program (k = the 128 hidden dims lives on partitions):
    tT_h [128k, 1024j]  = W1_txt^T-matmul of sign(txt)      (bf16, SBUF)
    aT   [128k, 128i]   = W1_img^T-matmul of sign(img) + b1 (f32, SBUF)
    per i (94 rows on VectorE at 4x bf16, 34 rows on ScalarE):
        r_i [128k, 1024j] = relu(tT_h + aT[:, i])           (bf16)
        for jb in 0..8:  # contiguous lhsT, negE lands transposed
            psum[jb//2][:, (jb%2)*128+i] = matmul(lhsT=r_i[:, jb*128:+128],
                                                  rhs=W2col)
    negO = exp(2*psum + 2*b2) in 3 i-phases -> [128jr, 8jb x 128i]  (ACT)
    out_pos = exp(posE/32), posE = sign-img x sign-txt matmul (exact bf16)
Host gathers: col0 = pos rows, col1 from negO via reshape/transpose, concat.
"""
import numpy as np
import ml_dtypes

N_CORES = 8
NI, NT, D, H = 1024, 1024, 64, 128
NI_LOC = NI // N_CORES  # 128
NJB = NT // H           # 8 psum column-blocks of 128 j
R_BUFS = 24             # in-flight relu tiles (DVE/ACT run-ahead over PE)

_compiled = None


ACT_NUM, ACT_DEN = 34, 128  # fraction of relu rows on ScalarE
GP_ROWS = 0              # relu rows on GpSimd (taken from VectorE's share)
SPLIT_ROWS = 6           # first rows emit half-width relu (start before full tT_h)


def _engine_map():
    """Per-i relu engine: 'A' (ScalarE), 'G' (GpSimd), 'V' (VectorE)."""
    eng = []
    acc = 0
    for i in range(NI_LOC):
        acc += ACT_NUM
        if acc >= ACT_DEN:
            acc -= ACT_DEN
            eng.append("A")
        else:
            eng.append("V")
    v_rows = [i for i in range(NI_LOC) if eng[i] == "V"]
    for n in range(GP_ROWS):
        eng[v_rows[(n * len(v_rows)) // GP_ROWS + 1]] = "G"
    return eng


def _build():
    import concourse.bacc as bacc
    import concourse.tile as tile
    import concourse.mybir as mybir

    F32 = mybir.dt.float32
    BF16 = mybir.dt.bfloat16
    AF = mybir.ActivationFunctionType
    ALU = mybir.AluOpType

    nc = bacc.Bacc("TRN2", target_bir_lowering=False, debug=False,
                   num_devices=N_CORES)

    txtT_d = nc.dram_tensor("txtT", [D, NT], BF16, kind="ExternalInput").ap()
    imgT_d = nc.dram_tensor("imgT", [D, NI_LOC], BF16, kind="ExternalInput").ap()
    wb64_d = nc.dram_tensor("wb64", [D, 2 * H], BF16, kind="ExternalInput").ap()
    wb128_d = nc.dram_tensor("wb128", [H, 3], F32, kind="ExternalInput").ap()
    pos_d = nc.dram_tensor("pos", [NI_LOC, NT], F32, kind="ExternalOutput").ap()
    # negO mirrors the on-chip layout: negO[jr, jb*128+i] = negE[i, jb*128+jr]
    negO_d = nc.dram_tensor("negO", [H, NT], F32, kind="ExternalOutput").ap()

    eng_map = _engine_map()
    CH = 512  # setup pipeline chunk

    with tile.TileContext(nc) as tc:
        with tc.tile_pool(name="const", bufs=1) as cpool, \
             tc.tile_pool(name="rp", bufs=R_BUFS) as rpool, \
             tc.tile_pool(name="op", bufs=1) as opool:

            # ---- trigger the ACT table load at t=0 (no input deps) -----------
            warm = cpool.tile([1, 1], F32)
            nc.vector.memset(warm[:], 0.0)
            nc.scalar.activation(warm[:], warm[:], AF.Exp, bias=0.0, scale=1.0)

            # ---- load inputs (bf16 hashes: sign() is scale-invariant) --------
            txtT_raw = cpool.tile([D, NT], BF16)
            nc.sync.dma_start(txtT_raw[:], txtT_d[:])
            imgT_raw = cpool.tile([D, NI_LOC], BF16)
            nc.sync.dma_start(imgT_raw[:], imgT_d[:])
            wb64 = cpool.tile([D, 2 * H], BF16)
            nc.sync.dma_start(wb64[:], wb64_d[:])
            wb128 = cpool.tile([H, 3], F32)
            nc.sync.dma_start(wb128[:], wb128_d[:])
            w1ti = wb64[:, 0:H]
            w1tt = wb64[:, H:2 * H]
            b1c = wb128[:, 0:1]
            b2s = wb128[:, 1:2]
            w2f = wb128[:, 2:3]
            w2c = cpool.tile([H, 1], BF16)
            nc.vector.tensor_copy(w2c[:], w2f)

            # ---- sign (+-1, bf16-exact), h-transforms, posE ------------------
            txtT_s = cpool.tile([D, NT], BF16)
            imgT_s = cpool.tile([D, NI_LOC], BF16)
            nc.scalar.activation(imgT_s[:], imgT_raw[:], AF.Sign)

            tT_h = cpool.tile([H, NT], BF16)
            aT = cpool.tile([H, NI_LOC], F32)
            pos_sb = opool.tile([NI_LOC, NT], F32)

            with tc.tile_pool(name="ps_set", bufs=2, space="PSUM") as ps_s, \
                 tc.tile_pool(name="ps_a", bufs=1, space="PSUM") as ps_a:
                aps = ps_a.tile([H, NI_LOC], F32)
                nc.tensor.matmul(aps[:], lhsT=w1ti, rhs=imgT_s[:],
                                 start=True, stop=True)
                nc.vector.tensor_scalar(aT[:], aps[:], b1c, None, op0=ALU.add)

                for hh in range(0, NT, CH):
                    nc.scalar.activation(txtT_s[:, hh:hh + CH],
                                         txtT_raw[:, hh:hh + CH], AF.Sign)
                    ps = ps_s.tile([H, CH], F32, tag="hps")
                    nc.tensor.matmul(ps[:], lhsT=w1tt,
                                     rhs=txtT_s[:, hh:hh + CH],
                                     start=True, stop=True)
                    nc.vector.tensor_copy(tT_h[:, hh:hh + CH], ps[:])

                for hh in range(0, NT, 512):
                    ps = ps_s.tile([NI_LOC, 512], F32, tag="pps")
                    nc.tensor.matmul(ps[:], lhsT=imgT_s[:],
                                     rhs=txtT_s[:, hh:hh + 512],
                                     start=True, stop=True)
                    nc.scalar.activation(pos_sb[:, hh:hh + 512], ps[:],
                                         AF.Exp, bias=0.0, scale=1.0 / 32.0)
            nc.sync.dma_start(pos_d[:], pos_sb[:])

            # ---- main pairwise loop (negE transposed: psum pair p holds
            #      jb=2p,2p+1 as [128j, 2*128i])
            with tc.tile_pool(name="ps_m", bufs=1, space="PSUM") as ps_m:
                psums = [ps_m.tile([H, 2 * NI_LOC], F32, tag=f"np{p}",
                                   name=f"negps{p}")
                         for p in range(NJB // 2)]
                negT_big = opool.tile([H, NT], F32)
                phases = [(0, 64), (64, 96), (96, NI_LOC)]
                HW_ = NT // 2
                for i0, i1 in phases:
                    for i in range(i0, i1):
                        if i < SPLIT_ROWS:
                            # two tiles so jb<4 matmuls only wait the lo half
                            r_lo = rpool.tile([H, HW_], BF16, tag="rlo")
                            r_hi = rpool.tile([H, HW_], BF16, tag="rhi")
                            parts = [(r_lo, 0), (r_hi, HW_)]
                        else:
                            r = rpool.tile([H, NT], BF16, tag="r")
                            parts = [(r, 0)]
                        for rt, off in parts:
                            w = HW_ if i < SPLIT_ROWS else NT
                            src = tT_h[:, off:off + w]
                            if eng_map[i] == "A":
                                nc.scalar.activation(rt[:], src, AF.Relu,
                                                     bias=aT[:, i:i + 1],
                                                     scale=1.0)
                            else:
                                nc.vector.tensor_scalar(rt[:], src,
                                                        aT[:, i:i + 1], 0.0,
                                                        op0=ALU.add,
                                                        op1=ALU.max)
                        for jb in range(NJB):
                            col = (jb % 2) * NI_LOC + i
                            if i < SPLIT_ROWS:
                                rt = parts[jb // 4][0]
                                lhsT = rt[:, (jb % 4) * H:(jb % 4 + 1) * H]
                            else:
                                lhsT = parts[0][0][:, jb * H:(jb + 1) * H]
                            nc.tensor.matmul(psums[jb // 2][:, col:col + 1],
                                             lhsT=lhsT,
                                             rhs=w2c[:], start=True, stop=True)
                    W = i1 - i0
                    for p in range(NJB // 2):
                        nc.scalar.activation(
                            negT_big[:, :].rearrange(
                                "j (p s i) -> j p s i",
                                p=NJB // 2, s=2)[:, p, :, i0:i1],
                            psums[p][:, :].rearrange(
                                "j (s i) -> j s i", s=2)[:, :, i0:i1],
                            AF.Exp, bias=b2s, scale=2.0)
                    nc.sync.dma_start(
                        negO_d[:, :].rearrange(
                            "j (jb i) -> j jb i", jb=NJB)[:, :, i0:i1],
                        negT_big[:, :].rearrange(
                            "j (jb i) -> j jb i", jb=NJB)[:, :, i0:i1])

    nc.compile()
    return nc


def _get_compiled():
    global _compiled
    if _compiled is None:
        _compiled = _build()
    return _compiled


def run(inputs: dict, trace: bool = False):
    """Shard, run on 8 cores, gather. Returns (full_output, BassKernelResults)."""
    from concourse.bass_utils import run_bass_kernel_spmd

    nc = _get_compiled()

    imgs = np.asarray(inputs["images_hash"], dtype=np.float32)
    txts = np.asarray(inputs["texts_hash"], dtype=np.float32)
    W1 = np.asarray(inputs["W1"], dtype=np.float32)
    b1 = np.asarray(inputs["b1"], dtype=np.float32)
    W2 = np.asarray(inputs["W2"], dtype=np.float32)
    b2 = np.asarray(inputs["b2"], dtype=np.float32)
    task = int(np.asarray(inputs["task_is_i2t"]))

    bf16 = ml_dtypes.bfloat16
    txtT = np.ascontiguousarray(txts.T).astype(bf16)                # [64, 1024]
    wb64 = np.concatenate(
        [W1[:, :D].T * 0.125, W1[:, D:].T * 0.125], axis=1).astype(bf16)
    wb128 = np.stack(
        [b1, np.full(H, 2.0 * float(b2[0]), np.float32), W2[0]],
        axis=1).astype(np.float32)

    in_maps = []
    for c in range(N_CORES):
        sl = imgs[c * NI_LOC:(c + 1) * NI_LOC]
        in_maps.append({
            "txtT": txtT,
            "imgT": np.ascontiguousarray(sl.T).astype(bf16),
            "wb64": wb64, "wb128": wb128,
        })

    res = run_bass_kernel_spmd(nc, in_maps, list(range(N_CORES)), trace=trace)

    full = np.empty((NI * NT, 2), dtype=np.float32)
    pos = np.concatenate([res.results[c]["pos"] for c in range(N_CORES)], axis=0)
    # negO[jr, jb*128+i] = negE[i, jb*128+jr]  ->  neg_core[i, j]
    neg = np.concatenate(
        [res.results[c]["negO"].reshape(H, NJB, NI_LOC).transpose(2, 1, 0)
         .reshape(NI_LOC, NT) for c in range(N_CORES)], axis=0)
    full[:, 0] = (pos if task else pos.T).reshape(-1)
    full[:, 1] = neg.reshape(-1)
    return full, res


def kernel(**inputs) -> np.ndarray:
    out, _ = run(inputs, trace=False)
    return out
